# revision 15
# baseline (speedup 1.0000x reference)
"""AxialAttention Trainium2 kernel.

The reference "axial attention" is actually a per-pixel op: at every pixel,
attention runs ACROSS THE 12 HEADS (12x12) with hd=64 contracted; the H-pass
and W-pass differ only in weights. So: data-parallel over the 8 batches
(one per NeuronCore), each core processes 4096 pixels:
  qkv = x @ W (fp32r matmuls, full rate at N>=256)
  per-pixel 12x12 head attention via block-diagonal Gram matmuls over
  groups of G=8 pixels (96x96 tiles), softmax via masked exp + ones-column
  denominator trick, all in bf16
  out = (attn_h + attn_w) @ W_proj + b_proj (fp32r)
"""
import sys, os
sys.path.insert(0, os.path.dirname(os.path.abspath(__file__)))
import numpy as np
import ml_dtypes

import concourse.bass as bass
import concourse.mybir as mybir
import concourse.tile as tile
from concourse.vector_clock import ScopedClock
from concourse.bass_utils import run_bass_kernel_spmd

# ---- walrus-compat patch: this env's walrus rejects >1 sync-wait per
# instruction; split the final tile drain into single-wait SP instructions.
def _patched_drain_and_barrier(self, tick_clock, wait_clock):
    nc = self.nc
    gc = tick_clock.global_clock
    for proc, sem in sorted(self.sems.allocated().items()):
        tick = gc[proc] if proc < len(gc) else 0
        if tick > 0:
            name = str(getattr(sem, 'name', '') or '')
            mult = 16 if 'DMA' in name else 1
            nc.sync.wait_ge(sem, tick * mult)
    nc.sync.drain()
    nc.all_engine_barrier()
    popped = nc._tile_sem_poison_stack.pop()
    assert popped is self._sem_poison
    nc.clear_and_free_semaphores(list(self.sems.allocated().values()))
    nc.all_engine_barrier()

tile.TileContext._drain_and_barrier = _patched_drain_and_barrier


def _split_multi_waits(nc):
    """This env's walrus accepts at most one sync-wait per instruction;
    hoist extra waits onto single-wait drains inserted just before."""
    cnt = 0
    for f in nc.m.functions:
        for bb in f.blocks:
            new_insts = []
            for inst in bb.instructions:
                si = inst.sync_info
                if si is not None and len(si.on_wait) > 1:
                    waits = list(si.on_wait)
                    for w in waits[:-1]:
                        cnt += 1
                        d = mybir.InstDrain(
                            name=f'WSPLIT-{cnt}',
                            engine=inst.engine,
                            ins=[], outs=[],
                            sync_info=mybir.SyncInfo(on_wait=[w],
                                                     on_update=[]))
                        nc.register_instruction(d, overwrite=True)
                        new_insts.append(d)
                    inst.sync_info = mybir.SyncInfo(
                        on_wait=[waits[-1]], on_update=list(si.on_update))
                new_insts.append(inst)
            bb.instructions = new_insts

F32 = mybir.dt.float32
F32R = mybir.dt.float32r
BF16 = mybir.dt.bfloat16

HEADS, HD, C = 12, 64, 768
PX = 4096            # pixels per core
BLK = 256            # pixels per block
NBLK = PX // BLK     # 16
G = 8                # pixels per attention group
NGRP = BLK // G      # 32 groups per block
GM = G * HEADS       # 96 = gram tile dim
SCALE = HD ** -0.5

_CACHED = {}


def _build():
    nc = bass.Bass(trn_type='TRN2')
    x_p = nc.dram_tensor('x_p', [PX, C], F32R, kind='ExternalInput')
    wqh = nc.dram_tensor('wqh', [128, 6, 3 * C], F32R, kind='ExternalInput')
    wqw = nc.dram_tensor('wqw', [128, 6, 3 * C], F32R, kind='ExternalInput')
    wp = nc.dram_tensor('wp', [128, 6, C], F32R, kind='ExternalInput')
    bqh = nc.dram_tensor('bqh', [128, 18], F32, kind='ExternalInput')
    bqw = nc.dram_tensor('bqw', [128, 18], F32, kind='ExternalInput')
    bp = nc.dram_tensor('bp', [128, 6], F32, kind='ExternalInput')
    mask = nc.dram_tensor('mask', [GM, GM], BF16, kind='ExternalInput')
    bqvh = nc.dram_tensor('bqvh', [128, 768], F32, kind='ExternalInput')
    bqvw = nc.dram_tensor('bqvw', [128, 768], F32, kind='ExternalInput')
    out_p = nc.dram_tensor('out_p', [PX, C], F32, kind='ExternalOutput')
    v_dram = nc.dram_tensor('v_dram', [BLK, C], BF16, kind='Internal')

    from contextlib import ExitStack
    with tile.TileContext(nc) as tc, ExitStack() as ctx:
        consts = ctx.enter_context(tc.tile_pool(name='consts', bufs=1))
        xt_pool = ctx.enter_context(tc.tile_pool(name='xt', bufs=2))
        qkvt_pool = ctx.enter_context(tc.tile_pool(name='qkvt', bufs=1))
        form_pool = ctx.enter_context(tc.tile_pool(name='form', bufs=1))
        attn_pool = ctx.enter_context(tc.tile_pool(name='attn', bufs=4))
        avs_pool = ctx.enter_context(tc.tile_pool(name='avs', bufs=1))
        prj_pool = ctx.enter_context(tc.tile_pool(name='prjp', bufs=1))
        ot_pool = ctx.enter_context(tc.tile_pool(name='otp', bufs=3))
        psum = ctx.enter_context(tc.tile_pool(name='psum', bufs=2, space='PSUM'))
        psum_s = ctx.enter_context(tc.tile_pool(name='psum_s', bufs=2, space='PSUM'))

        wqh_sb = consts.tile([128, 6, 3 * C], F32R)
        wqw_sb = consts.tile([128, 6, 3 * C], F32R)
        wp_sb = consts.tile([128, 6, C], F32R)
        bqh_sb = consts.tile([128, 18], F32)
        bqw_sb = consts.tile([128, 18], F32)
        bp_sb = consts.tile([128, 6], F32)
        mask_sb = consts.tile([GM, GM], BF16)
        bqv_h_sb = consts.tile([128, 768], F32)
        ones_sb = consts.tile([1, 64], F32)
        bqv_w_sb = consts.tile([128, 768], F32)
        nc.sync.dma_start(wqh_sb[:], wqh[:])
        nc.sync.dma_start(wqw_sb[:], wqw[:])
        nc.sync.dma_start(wp_sb[:], wp[:])
        nc.sync.dma_start(bqh_sb[:], bqh[:])
        nc.sync.dma_start(bqw_sb[:], bqw[:])
        nc.sync.dma_start(bp_sb[:], bp[:])
        nc.sync.dma_start(mask_sb[:], mask[:])
        nc.sync.dma_start(bqv_h_sb[:], bqvh[:])
        nc.any.memset(ones_sb[:], 1.0)
        nc.sync.dma_start(bqv_w_sb[:], bqvw[:])

        for blk in range(NBLK):
            px0 = blk * BLK
            # --- transpose-load Xt [128, 6, BLK] (c-major) ---
            xt = xt_pool.tile([128, 6, BLK], F32R, tag='xt')
            with nc.allow_non_contiguous_dma(reason='transpose load'):
                for kt in range(6):
                    nc.sync.dma_start(
                        xt[:, kt, :],
                        x_p[px0:px0 + BLK, kt * 128:(kt + 1) * 128]
                        .rearrange('n k -> k n'))

            avsum = avs_pool.tile([64, NGRP, GM], F32R, tag='avsum')

            for ps_i, (wq_sb, bq_sb, bqv_sb) in enumerate(
                    [(wqh_sb, bqh_sb, bqv_h_sb), (wqw_sb, bqw_sb, bqv_w_sb)]):
                # --- QKV^T: 18 m-tiles x 6 k-tiles, fp32r, bf16 out ---
                qkvt = qkvt_pool.tile([128, 12, BLK], BF16, tag='qkvt')
                for mt in range(12):
                    ps = psum.tile([128, BLK], F32, tag='mm_ps')
                    for kt in range(6):
                        nc.tensor.matmul(
                            ps[:],
                            lhsT=wq_sb[:, kt, mt * 128:(mt + 1) * 128],
                            rhs=xt[:, kt, :],
                            start=(kt == 0), stop=(kt == 5))
                    nc.scalar.activation(
                        qkvt[:, mt, :], ps[:],
                        mybir.ActivationFunctionType.Identity,
                        bias=bq_sb[:, mt:mt + 1], scale=1.0)

                # --- V pixel-major: swapped-operand matmuls ---
                v_px = form_pool.tile([128, 2, C], BF16, tag='vpx')
                for ph in range(2):
                    for nci in range(2):
                        n0 = 1536 + nci * 384
                        ps_v = psum.tile([128, 384], F32, tag='mm_ps')
                        for kt in range(6):
                            nc.tensor.matmul(
                                ps_v[:],
                                lhsT=xt[:, kt, ph * 128:(ph + 1) * 128],
                                rhs=wq_sb[:, kt, n0:n0 + 384],
                                start=(kt == 0), stop=(kt == 5))
                        nc.vector.tensor_add(
                            v_px[:, ph, nci * 384:(nci + 1) * 384], ps_v[:],
                            bqv_sb[:, nci * 384:(nci + 1) * 384])
                # --- forms: per-group-contiguous column layouts ---
                qf = form_pool.tile([64, NGRP, GM], BF16, tag='qf')
                kf = form_pool.tile([64, NGRP, GM], BF16, tag='kf')
                vf = form_pool.tile([GM, NGRP, 65], BF16, tag='vf')
                with nc.allow_non_contiguous_dma(reason='form gather'):
                    for e in range(2):
                        for j in range(6):
                            h = 2 * j + e
                            src_q = (qkvt[e * 64:(e + 1) * 64, j, :]
                                     .rearrange('d (grp p) -> d grp p', p=G))
                            nc.sync.dma_start(
                                qf[:, :, h * G:(h + 1) * G], src_q)
                            src_k = (qkvt[e * 64:(e + 1) * 64, 6 + j, :]
                                     .rearrange('d (grp p) -> d grp p', p=G))
                            nc.sync.dma_start(
                                kf[:, :, h * G:(h + 1) * G], src_k)
                    # V bounce through DRAM (pixel-major) to cross partitions
                    for ph in range(2):
                        nc.sync.dma_start(
                            v_dram[ph * 128:(ph + 1) * 128, :],
                            v_px[:, ph, :])
                    for g in range(HEADS):
                        # vf[g*8+p, grp, d] = v_dram[grp*8+p, g*64+d]
                        src = (v_dram[:]
                               .rearrange('(grp p) (g d) -> g p grp d', p=G,
                                          d=HD)[g])
                        nc.sync.dma_start(
                            vf[g * G:(g + 1) * G, :, 0:64], src)
                nc.any.memset(vf[:, :, 64:65], 1.0)

                for grp in range(NGRP):
                    st_ps = psum_s.tile([GM, GM], F32, tag='st')
                    nc.tensor.matmul(st_ps[:], lhsT=kf[:, grp, :],
                                     rhs=qf[:, grp, :], start=True, stop=True)
                    e_sb = attn_pool.tile([GM, GM], BF16, tag='e')
                    nc.scalar.activation(
                        e_sb[:], st_ps[:],
                        mybir.ActivationFunctionType.Exp, scale=SCALE)
                    em_sb = attn_pool.tile([GM, GM], BF16, tag='em')
                    nc.vector.tensor_mul(em_sb[:], e_sb[:], mask_sb[:])
                    av_ps = psum_s.tile([65, GM], F32, tag='av')
                    nc.tensor.matmul(av_ps[:], lhsT=vf[:, grp, :],
                                     rhs=em_sb[:], start=True, stop=True)
                    rd = attn_pool.tile([1, GM], F32, tag='rd')
                    nc.vector.reciprocal(rd[:], av_ps[64:65, :])
                    rdb_ps = psum_s.tile([64, GM], F32, tag='av')
                    nc.tensor.matmul(rdb_ps[:], lhsT=ones_sb[:], rhs=rd[:],
                                     start=True, stop=True)
                    rdb = attn_pool.tile([64, GM], F32, tag='rdbs')
                    nc.scalar.copy(rdb[:], rdb_ps[:])
                    if ps_i == 0:
                        nc.vector.tensor_mul(avsum[:, grp, :],
                                             av_ps[0:64, :], rdb[:])
                    else:
                        tmp = attn_pool.tile([64, GM], F32, tag='avtmp')
                        nc.vector.tensor_mul(tmp[:], av_ps[0:64, :], rdb[:])
                        nc.vector.tensor_add(avsum[:, grp, :],
                                             avsum[:, grp, :], tmp[:])

            # --- merge (h-parity) into proj rhs [128, 6, BLK] ---
            prj = prj_pool.tile([128, 6, BLK], F32R, tag='prj')
            with nc.allow_non_contiguous_dma(reason='parity merge'):
                for e in range(2):
                    for j in range(6):
                        h = 2 * j + e
                        nc.sync.dma_start(
                            prj[e * 64:(e + 1) * 64, j, :],
                            avsum[:, :, h * G:(h + 1) * G])

            # --- proj + bias + store ---
            for mt in range(6):
                ps = psum.tile([128, BLK], F32, tag='mm_ps')
                for kt in range(6):
                    nc.tensor.matmul(
                        ps[:],
                        lhsT=wp_sb[:, kt, mt * 128:(mt + 1) * 128],
                        rhs=prj[:, kt, :],
                        start=(kt == 0), stop=(kt == 5))
                ot = ot_pool.tile([128, BLK], F32, tag='ot')
                nc.scalar.activation(
                    ot[:], ps[:], mybir.ActivationFunctionType.Identity,
                    bias=bp_sb[:, mt:mt + 1], scale=1.0)
                with nc.allow_non_contiguous_dma(reason='transpose store'):
                    nc.sync.dma_start(
                        out_p[px0:px0 + BLK, mt * 128:(mt + 1) * 128]
                        .rearrange('n k -> k n'), ot[:])
    _split_multi_waits(nc)
    return nc


def _prep_inputs(x, w_qkv_h, b_qkv_h, w_qkv_w, b_qkv_w, w_proj, b_proj):
    wqh = np.ascontiguousarray(
        w_qkv_h.reshape(6, 128, 3 * C).transpose(1, 0, 2))
    wqw = np.ascontiguousarray(
        w_qkv_w.reshape(6, 128, 3 * C).transpose(1, 0, 2))
    wp = np.ascontiguousarray(w_proj.reshape(6, 128, C).transpose(1, 0, 2))
    bqh = np.ascontiguousarray(b_qkv_h.reshape(18, 128).T)
    bqw = np.ascontiguousarray(b_qkv_w.reshape(18, 128).T)
    bqvh = np.ascontiguousarray(
        np.broadcast_to(b_qkv_h[None, 1536:2304], (128, 768)))
    bqvw = np.ascontiguousarray(
        np.broadcast_to(b_qkv_w[None, 1536:2304], (128, 768)))
    bpr = np.ascontiguousarray(b_proj.reshape(6, 128).T)
    m = np.zeros((GM, GM), np.float32)
    idx = np.arange(GM)
    m[(idx[:, None] % G) == (idx[None, :] % G)] = 1.0
    m = m.astype(ml_dtypes.bfloat16)
    shared = dict(wqh=wqh, wqw=wqw, wp=wp, bqh=bqh, bqw=bqw, bp=bpr,
              mask=m, bqvh=bqvh, bqvw=bqvw)
    B = x.shape[0]
    maps = []
    for i in range(B):
        d = dict(shared)
        d['x_p'] = np.ascontiguousarray(x[i].reshape(PX, C))
        maps.append(d)
    return maps


def kernel(x, w_qkv_h, b_qkv_h, w_qkv_w, b_qkv_w, w_proj, b_proj):
    x = np.asarray(x, np.float32)
    if 'nc' not in _CACHED:
        _CACHED['nc'] = _build()
    nc = _CACHED['nc']
    in_maps = _prep_inputs(np.asarray(x, np.float32),
                           np.asarray(w_qkv_h, np.float32),
                           np.asarray(b_qkv_h, np.float32),
                           np.asarray(w_qkv_w, np.float32),
                           np.asarray(b_qkv_w, np.float32),
                           np.asarray(w_proj, np.float32),
                           np.asarray(b_proj, np.float32))
    try:
        res = run_bass_kernel_spmd(nc, in_maps, core_ids=list(range(8)),
                                   trace=bool(os.environ.get('KTRACE')))
    except ModuleNotFoundError:
        res = run_bass_kernel_spmd(nc, in_maps, core_ids=list(range(8)))
    _CACHED['last_result'] = res
    B, H, W, _ = x.shape
    out = np.stack([res.results[i]['out_p'].reshape(H, W, C)
                    for i in range(B)])
    return out
